# revision 10
# baseline (speedup 1.0000x reference)
"""Trainium2 Bass kernel for nn_DeformableRead (deformable attention read).

8 NeuronCores SPMD: core q -> batch q//4, anchor-cell rows 8*(q%4)..+8 (256
cells). Tokens routed to the core owning their anchor cell and densely packed
into 8-slot groups (a cell with n tokens occupies ceil(n/8) groups); 16 groups
per 128-slot chunk. Sample points live in fixed windows around each anchor
cell (9x9/5x5/4x4 at L2/L3/L4); bilinear sampling over a window is a dense
122-tap PE contraction (zero-padded to 128 taps) with separable hat weights
relu(1-|xi-i|) -- gather-free. Offsets/logits are produced row-major
(slot-partition) via a transposed wda matmul with LN affine + biases folded
into the weights; softmax and hat/kappa construction split across
Vector/GpSimd/Scalar engines. Host does layout only.
"""

import numpy as np
import ml_dtypes

import concourse.bass as bass
import concourse.bacc as bacc
import concourse.tile as tile
from concourse import mybir
from concourse.bass_utils import run_bass_kernel_spmd

D, H, NL, M = 192, 6, 3, 4
NF = 8
SIGMAS = (4.0, 2.0, 1.0)
WXY = (9, 5, 4)
CLO = (4.0, 2.0, 1.5)
PADL = (2, 1, 1)
SCALE = (4, 2, 1)
G8 = 8                    # slots per group
CPC = 16                  # groups per chunk
NCH = 24                  # chunks per core (asserted in host prep)
S = NCH * 128             # 3072 slots
NCS = S // 512            # phase-A chunks
KW = 128                  # padded tap count
WSQ = (81, 25, 16)
WSQ2 = (82, 26, 16)       # even-padded tap blocks
LOFF2 = (0, 82, 108)      # tap offset per level in padded 128-tap space
TOFF2 = (0, 6 * 4 * 82, 6 * 4 * 82 + 6 * 4 * 26)   # (0, 1968, 2592)
TMPW = TOFF2[2] + 6 * 4 * 16                        # 2976
SOFF2 = (0, 6 * 2 * 82, 6 * 2 * 82 + 6 * 2 * 26)   # (0, 984, 1296)
SUMW = SOFF2[2] + 6 * 2 * 16                        # 1488
HOFF = (0, 36, 56)        # per-head hat sub-block offsets (x or y block)
HATB = 432                # per-coord hat block (6h * 72)
BF16 = mybir.dt.bfloat16
F32 = mybir.dt.float32

_CACHE = {}


def _ap(base, free_off, dims):
    """Custom AP: base tile slice (sets partition range), explicit free dims."""
    return bass.AP(tensor=base.tensor, offset=base.offset + free_off,
                   ap=[base.ap[0]] + [list(d) for d in dims])


def _build_module():
    nc = bacc.Bacc("TRN2", target_bir_lowering=False, debug=False)
    dt = nc.dram_tensor
    uinT = dt("uinT", [2 * D + 32, S], BF16, kind="ExternalInput")
    pblob = dt("pblob", [NCH, KW, CPC * D], BF16, kind="ExternalInput")
    wu = dt("wu", [2 * D + 32, D], BF16, kind="ExternalInput")
    wub = dt("wub", [D, 1], F32, kind="ExternalInput")
    wda = dt("wda", [D + 1, 216], BF16, kind="ExternalInput")
    sigc = dt("sigc", [128, 144], F32, kind="ExternalInput")
    iota2 = dt("iota2", [128, 2 * HATB], F32, kind="ExternalInput")
    onesw = dt("onesw", [96, 96], BF16, kind="ExternalInput")
    identb = dt("identb", [128, 128], BF16, kind="ExternalInput")
    wo1 = dt("wo1", [128, D], BF16, kind="ExternalInput")
    wo2 = dt("wo2", [64, D], BF16, kind="ExternalInput")
    bo = dt("bo", [D, 1], F32, kind="ExternalInput")
    outT = dt("outT", [D, S], F32, kind="ExternalOutput")

    AF = mybir.ActivationFunctionType
    OP = mybir.AluOpType
    AX = mybir.AxisListType
    V = None  # engine markers resolved below

    with tile.TileContext(nc) as tc:
        with (
            tc.tile_pool(name="const", bufs=1) as cpool,
            tc.tile_pool(name="big", bufs=1) as bpool,
        ):
            _sbn = [0]
            def sb(t_ap, shape, dtype):
                _sbn[0] += 1
                nm = f"cst{_sbn[0]}"
                x = cpool.tile(shape, dtype, tag=nm, name=nm)
                nc.sync.dma_start(x[:], t_ap)
                return x

            s_wu = []
            for kc in range(4):
                k0, k1 = kc * 128, min((kc + 1) * 128, 416)
                s_wu.append(sb(wu[k0:k1, :], [k1 - k0, D], BF16))
            s_wub = [sb(wub[0:96, :], [96, 1], F32), sb(wub[96:192, :], [96, 1], F32)]
            s_wda0 = sb(wda[0:96, :], [96, 216], BF16)
            s_wda1 = sb(wda[96:193, :], [97, 216], BF16)
            s_sigc = sb(sigc[:], [128, 144], F32)
            s_iota = sb(iota2[:], [128, 2 * HATB], F32)
            s_ones = sb(onesw[:], [96, 96], BF16)
            s_idb = sb(identb[:], [128, 128], BF16)
            s_wo1 = sb(wo1[:], [128, D], BF16)
            s_wo2 = sb(wo2[:], [64, D], BF16)
            s_bo = [sb(bo[0:96, :], [96, 1], F32), sb(bo[96:192, :], [96, 1], F32)]
            s_eps = cpool.tile([96, 1], F32, name="s_eps")
            nc.vector.memset(s_eps[:], 1e-5)

            u_r = [bpool.tile([96, S], BF16, tag="ur0", name="ur0"),
                   bpool.tile([97, S], BF16, tag="ur1", name="ur1")]
            nc.vector.memset(u_r[1][96:97, :], 1.0)
            ymS = [bpool.tile([96, S], BF16, tag="ym0", name="ym0"),
                   bpool.tile([96, S], BF16, tag="ym1", name="ym1")]
            varS = bpool.tile([96, S], F32, tag="varS", name="varS")
            rrS = bpool.tile([96, S], BF16, tag="rrS", name="rrS")

            # ======== phase A: u = gelu(W_u @ u_in); LN stats ========
            with (
                tc.tile_pool(name="wk", bufs=2) as wpool,
                tc.tile_pool(name="ucp", bufs=2) as ucpool,
                tc.tile_pool(name="psU", bufs=2, space="PSUM") as psU,
                tc.tile_pool(name="psB", bufs=2, space="PSUM") as psB,
            ):
                for ci in range(NCS):
                    n0 = ci * 512
                    uc = ucpool.tile([128, 4, 512], BF16, tag="uc")
                    for kc in range(4):
                        k0, k1 = kc * 128, min((kc + 1) * 128, 416)
                        nc.sync.dma_start(uc[:k1 - k0, kc, :],
                                          uinT[k0:k1, n0:n0 + 512])
                    y = []
                    for mc in range(2):
                        pu = psU.tile([96, 512], F32, tag="pu")
                        for kc in range(4):
                            kk = min(128, 416 - kc * 128)
                            nc.tensor.matmul(
                                pu[:],
                                s_wu[kc][:, mc * 96:(mc + 1) * 96],
                                uc[:kk, kc, :],
                                start=(kc == 0), stop=(kc == 3))
                        yt = wpool.tile([96, 512], BF16, tag=f"y{mc}", name=f"y{mc}")
                        nc.scalar.activation(out=yt[:], in_=pu[:],
                                             func=AF.Gelu, bias=s_wub[mc],
                                             scale=1.0)
                        y.append(yt)
                    y2 = []
                    for mc in range(2):
                        y2t = wpool.tile([96, 512], BF16, tag=f"y2{mc}", name=f"y2{mc}")
                        nc.vector.tensor_mul(y2t[:], y[mc][:], y[mc][:])
                        y2.append(y2t)
                    pst = psB.tile([96, 2, 512], F32, tag="pst")
                    for st, srcs in ((0, y), (1, y2)):
                        for kc in range(2):
                            nc.tensor.matmul(
                                pst[:, st, :], s_ones[:], srcs[kc][:],
                                start=(kc == 0), stop=(kc == 1))
                    mu = wpool.tile([96, 512], F32, tag="mu")
                    nc.scalar.mul(mu[:], pst[:, 0, :], 1.0 / D)
                    mu2 = wpool.tile([96, 512], F32, tag="mu2")
                    nc.scalar.activation(out=mu2[:], in_=pst[:, 0, :],
                                         func=AF.Square, scale=1.0 / D)
                    nc.vector.scalar_tensor_tensor(
                        out=varS[:, n0:n0 + 512], in0=pst[:, 1, :],
                        scalar=1.0 / D, in1=mu2[:],
                        op0=OP.mult, op1=OP.subtract)
                    for mc, eng in ((0, nc.gpsimd), (1, nc.vector)):
                        eng.tensor_sub(ymS[mc][:, n0:n0 + 512],
                                       y[mc][:], mu[:])
                # LN epilogue: rr = 1/sqrt(var+eps); u_r = ym * rr
                nc.scalar.activation(out=rrS[:], in_=varS[:],
                                     func=AF.Abs_reciprocal_sqrt,
                                     bias=s_eps, scale=1.0)
                for mc in range(2):
                    nc.vector.tensor_mul(u_r[mc][0:96, :], ymS[mc][:], rrS[:])

            # ======== phase B: per 128-slot chunk ========
            with (
                tc.tile_pool(name="fp", bufs=3) as fpool,
                tc.tile_pool(name="tp", bufs=3) as tpool,
                tc.tile_pool(name="kp", bufs=3) as kpool,
                tc.tile_pool(name="pp", bufs=2) as ppool,
                tc.tile_pool(name="psA", bufs=1, space="PSUM") as psA,
                tc.tile_pool(name="psK", bufs=2, space="PSUM") as psK,
                tc.tile_pool(name="psX", bufs=1, space="PSUM") as psX,
                tc.tile_pool(name="psD", bufs=2, space="PSUM") as psD,
            ):
                KB = 3  # kpool bufs
                for q in range(NCH):
                    c0 = q * 128
                    # -- offsets/logits row-major --
                    pda = psA.tile([128, 216], F32, tag="pda")
                    nc.tensor.matmul(pda[:], u_r[0][:, c0:c0 + 128],
                                     s_wda0[:], start=True, stop=False)
                    nc.tensor.matmul(pda[:], u_r[1][:, c0:c0 + 128],
                                     s_wda1[:], start=False, stop=True)
                    th = fpool.tile([128, 144], F32, tag="th")
                    nc.scalar.activation(out=th[:], in_=pda[:, 0:144],
                                         func=AF.Tanh)
                    ex = fpool.tile([128, 72], F32, tag="ex")
                    nc.scalar.activation(out=ex[:], in_=pda[:, 144:216],
                                         func=AF.Exp)
                    ssum = fpool.tile([128, 6], F32, tag="ssum")
                    nc.vector.tensor_reduce(
                        out=ssum[:], in_=_ap(ex[:], 0, [[12, 6], [1, 12]]),
                        axis=AX.X, op=OP.add)
                    rz = fpool.tile([128, 6], F32, tag="rz")
                    nc.vector.reciprocal_approx_fast(out=rz[:], in_=ssum[:])
                    wts = fpool.tile([128, 72], F32, tag="wts")
                    nc.vector.tensor_mul(wts[:], ex[:],
                                         _ap(rz[:], 0, [[1, 6], [0, 12]]))
                    # -- hats --
                    rm2 = fpool.tile([128, 144], F32, tag="rm2")
                    nc.vector.tensor_mul(rm2[:], th[:], s_sigc[:])
                    hxy = fpool.tile([128, 2 * HATB], F32, tag="hxy")
                    for coord, l, eng in ((0, 0, nc.gpsimd), (0, 1, nc.gpsimd),
                                          (0, 2, nc.vector), (1, 0, nc.gpsimd),
                                          (1, 1, nc.gpsimd), (1, 2, nc.vector)):
                        w = WXY[l]
                        eng.tensor_sub(
                            _ap(hxy[:], coord * HATB + HOFF[l],
                                [[72, 6], [w, 4], [1, w]]),
                            _ap(rm2[:], 8 * l + coord,
                                [[24, 6], [2, 4], [0, w]]),
                            _ap(s_iota[:], coord * HATB + HOFF[l],
                                [[72, 6], [w, 4], [1, w]]))
                    nc.scalar.activation(out=hxy[:], in_=hxy[:], func=AF.Abs)
                    nc.scalar.activation(out=hxy[:], in_=hxy[:], func=AF.Relu,
                                         bias=1.0, scale=-1.0)
                    # -- weighted y-hats --
                    hyw = fpool.tile([128, HATB], F32, tag="hyw")
                    for l, eng in ((0, nc.gpsimd), (1, nc.vector),
                                   (2, nc.vector)):
                        w = WXY[l]
                        eng.tensor_mul(
                            _ap(hyw[:], HOFF[l], [[72, 6], [w, 4], [1, w]]),
                            _ap(hxy[:], HATB + HOFF[l],
                                [[72, 6], [w, 4], [1, w]]),
                            _ap(wts[:], 4 * l, [[12, 6], [1, 4], [0, w]]))
                    # -- products --
                    tmp = tpool.tile([128, TMPW], BF16, tag="tmp")
                    for l in range(NL):
                        w = WXY[l]
                        for m in range(4):
                            eng = nc.gpsimd if (l == 0 and m % 2 == 1) else nc.vector
                            eng.tensor_mul(
                                _ap(tmp[:], TOFF2[l] + m * WSQ2[l],
                                    [[4 * WSQ2[l], 6], [w, w], [1, w]]),
                                _ap(hyw[:], HOFF[l] + m * w,
                                    [[72, 6], [1, w], [0, w]]),
                                _ap(hxy[:], HOFF[l] + m * w,
                                    [[72, 6], [0, w], [1, w]]))
                    # -- pairwise adds --
                    sums = fpool.tile([128, SUMW], BF16, tag="sums")
                    for l, eng in ((0, nc.gpsimd), (1, nc.vector),
                                   (2, nc.gpsimd)):
                        w2, w2p = WSQ[l], WSQ2[l]
                        eng.tensor_add(
                            _ap(sums[:], SOFF2[l],
                                [[2 * w2p, 6], [w2p, 2], [1, w2]]),
                            _ap(tmp[:], TOFF2[l],
                                [[4 * w2p, 6], [2 * w2p, 2], [1, w2]]),
                            _ap(tmp[:], TOFF2[l] + w2p,
                                [[4 * w2p, 6], [2 * w2p, 2], [1, w2]]))
                    kap = kpool.tile([128, 6 * KW], BF16, tag="kap")
                    if q < KB:
                        nc.vector.memset(kap[:], 0.0)
                    for l, eng in ((0, nc.gpsimd), (1, nc.vector),
                                   (2, nc.vector)):
                        w2, w2p = WSQ[l], WSQ2[l]
                        eng.tensor_add(
                            _ap(kap[:], LOFF2[l], [[KW, 6], [1, w2]]),
                            _ap(sums[:], SOFF2[l], [[2 * w2p, 6], [1, w2]]),
                            _ap(sums[:], SOFF2[l] + w2p,
                                [[2 * w2p, 6], [1, w2]]))
                    # -- transpose kappa per head -> [tap, h, slot] --
                    pK = psK.tile([KW, 6, 128], BF16, tag="pK")
                    for hh in range(H):
                        nc.tensor.transpose(pK[:, hh, :],
                                            kap[:, hh * KW:(hh + 1) * KW],
                                            s_idb[:])
                    kT = kpool.tile([KW, 6, 128], BF16, tag="kT")
                    nc.scalar.copy(out=kT[:, 0:3, :], in_=pK[:, 0:3, :])
                    nc.scalar.copy(out=kT[:, 3:6, :], in_=pK[:, 3:6, :])
                    # -- patch sampling matmuls --
                    patch = ppool.tile([KW, CPC * D], BF16, tag="patch")
                    nc.sync.dma_start(patch[:], pblob[q])
                    pX = psX.tile([128, 1024], F32, tag="pX")
                    pXc = psX.tile([64, 256], F32, tag="pXc")
                    for gg in range(CPC):
                        xo = (gg // 8) * 512 + (gg % 8) * 48
                        nc.tensor.matmul(
                            pX[:, xo:xo + 48],
                            patch[:, gg * D:gg * D + 128],
                            _ap(kT[:], gg * G8, [[128, 6], [1, G8]]),
                            start=True, stop=True)
                        nc.tensor.matmul(
                            pXc[:, gg * 16:(gg + 1) * 16],
                            patch[:, gg * D + 128:(gg + 1) * D],
                            _ap(kT[:], 4 * 128 + gg * G8, [[128, 2], [1, G8]]),
                            start=True, stop=True)
                    # -- head-diagonal selection PSUM->SBUF --
                    XU = kpool.tile([128, 128], BF16, tag="XU")
                    for hh in range(4):
                        src = _ap(pX[32 * hh:32 * hh + 32, :], hh * G8,
                                  [[512, 2], [48, 8], [1, G8]])
                        dst = _ap(XU[32 * hh:32 * hh + 32, :], 0,
                                  [[64, 2], [8, 8], [1, G8]])
                        nc.scalar.copy(out=dst, in_=src)
                    XL = kpool.tile([64, 128], BF16, tag="XL")
                    for h2 in range(2):
                        src = _ap(pXc[32 * h2:32 * h2 + 32, :], h2 * G8,
                                  [[16, 16], [1, G8]])
                        dst = _ap(XL[32 * h2:32 * h2 + 32, :], 0,
                                  [[8, 16], [1, G8]])
                        nc.scalar.copy(out=dst, in_=src)
                    # -- output projection --
                    for mc in range(2):
                        pD = psD.tile([96, 128], F32, tag="pD")
                        nc.tensor.matmul(pD[:], s_wo1[:, mc * 96:(mc + 1) * 96],
                                         XU[:], start=True, stop=False)
                        nc.tensor.matmul(pD[:], s_wo2[:, mc * 96:(mc + 1) * 96],
                                         XL[:], start=False, stop=True)
                        od = kpool.tile([96, 128], F32, tag=f"od{mc}",
                                        name=f"od{mc}")
                        nc.scalar.activation(out=od[:], in_=pD[:],
                                             func=AF.Identity, bias=s_bo[mc],
                                             scale=1.0)
                        nc.sync.dma_start(
                            outT[mc * 96:(mc + 1) * 96, c0:c0 + 128], od[:])
    nc.compile()
    return nc


def _host_prep(inputs):
    h = inputs["h"].astype(np.float32)
    ti = inputs["top_indices"].astype(np.int64)
    qc = inputs["query_coords"].astype(np.float32)
    g = inputs["g"].astype(np.float32)
    maps = [np.asarray(inputs["L2_proj"], np.float32),
            np.asarray(inputs["L3_proj"], np.float32),
            np.asarray(inputs["L4_proj"], np.float32)]
    B, K, R = ti.shape

    consts = {}
    consts["wu"] = np.ascontiguousarray(inputs["w_u_w"].T).astype(ml_dtypes.bfloat16)
    consts["wub"] = inputs["w_u_b"].reshape(D, 1).astype(np.float32)
    # wda: [193, 216]; rows 0:192 = (concat(delta, logit) * ln_g).T; row 192 =
    # W @ ln_b + layer bias (consumed via the ones-row of u_r)
    wrows = np.concatenate([inputs["w_delta_w"], inputs["w_a_w"]], 0)  # [216,192]
    brow = (wrows @ inputs["ln_u_b"]
            + np.concatenate([inputs["w_delta_b"], inputs["w_a_b"]]))
    wda = np.concatenate([(wrows * inputs["ln_u_g"][None, :]).T,
                          brow[None, :]], 0)
    consts["wda"] = wda.astype(ml_dtypes.bfloat16)
    sig = np.zeros((H, NL, M, 2), np.float32)
    for l in range(NL):
        sig[:, l] = SIGMAS[l]
    consts["sigc"] = np.tile(sig.reshape(1, 144), (128, 1))
    io = np.zeros((128, 2 * HATB), np.float32)
    for coord in range(2):
        for l in range(NL):
            w = WXY[l]
            for hh in range(H):
                for m in range(M):
                    st = coord * HATB + HOFF[l] + 72 * hh + w * m
                    io[:, st:st + w] = np.arange(w, dtype=np.float32) - CLO[l]
    consts["iota2"] = io
    consts["onesw"] = np.ones((96, 96), ml_dtypes.bfloat16)
    consts["identb"] = np.eye(128, dtype=ml_dtypes.bfloat16)
    woT = np.ascontiguousarray(inputs["w_o_w"].T).astype(np.float32)
    consts["wo1"] = woT[0:128].astype(ml_dtypes.bfloat16)
    consts["wo2"] = woT[128:192].astype(ml_dtypes.bfloat16)
    consts["bo"] = (inputs["w_o_b"] + inputs["e_deform"].reshape(-1)).reshape(D, 1).astype(np.float32)

    pmaps = []
    for b in range(B):
        pm = []
        for l in range(NL):
            Wl = maps[l].shape[3]
            mp = np.transpose(maps[l][b], (1, 2, 0))
            Hp = 32 * SCALE[l] + WXY[l]
            out = np.zeros((Hp, Hp, D), np.float32)
            out[PADL[l]:PADL[l] + Wl, PADL[l]:PADL[l] + Wl] = mp
            pm.append(out)
        pmaps.append(pm)

    freqs = 2.0 ** np.arange(NF, dtype=np.float32)
    NG = NCH * CPC

    in_maps, slot_maps = [], []
    for q in range(8):
        b, crow = q // 4, q % 4
        d = dict(consts)
        cell_of = ti[b].reshape(K * R)
        gcell = np.zeros(NG, np.int64)
        slot_tok = -np.ones(S, np.int64)
        gi = 0
        for cid in range(256):
            toks = np.nonzero(cell_of == crow * 256 + cid)[0]
            for j0 in range(0, len(toks), G8):
                sub = toks[j0:j0 + G8]
                assert gi < NG, "group overflow; raise NCH"
                gcell[gi] = cid
                slot_tok[gi * G8:gi * G8 + len(sub)] = sub
                gi += 1
        # patch blob [NCH, 128, CPC*D]
        ay, ax = gcell // 32, gcell % 32
        pats = np.zeros((NG, KW, D), np.float32)
        for l in range(NL):
            w = WXY[l]
            pm = pmaps[b][l]
            ys = (SCALE[l] * 8 * crow + SCALE[l] * ay)[:, None] + np.arange(w)
            xs = (SCALE[l] * ax)[:, None] + np.arange(w)
            pt = pm[ys[:, :, None], xs[:, None, :], :]      # [NG,w,w,D]
            pats[:, LOFF2[l]:LOFF2[l] + w * w] = pt.reshape(NG, w * w, D)
        d["pblob"] = np.ascontiguousarray(
            pats.reshape(NCH, CPC, KW, D).transpose(0, 2, 1, 3)
                .reshape(NCH, KW, CPC * D)).astype(ml_dtypes.bfloat16)
        valid = slot_tok >= 0
        st = np.where(valid, slot_tok, 0)
        k_of = st // R
        cid_of = cell_of[st]
        h_s = h[b][k_of] * valid[:, None]
        g_s = g[b][cid_of] * valid[:, None]
        qc_s = qc[b][k_of]
        axg = (cid_of % 32).astype(np.float32)
        ayg = (cid_of // 32).astype(np.float32)
        anchor = np.stack([axg * 32 + 16, ayg * 32 + 16], -1)
        dp = (anchor - qc_s) / 1024.0
        xf = dp[:, 0:1] * freqs * 2 * np.pi
        yf = dp[:, 1:2] * freqs * 2 * np.pi
        phi = np.concatenate([np.sin(xf), np.cos(xf), np.sin(yf), np.cos(yf)],
                             -1).astype(np.float32) * valid[:, None]
        u_in = np.concatenate([h_s, g_s, phi], -1)
        d["uinT"] = np.ascontiguousarray(u_in.T).astype(ml_dtypes.bfloat16)
        in_maps.append(d)
        slot_maps.append((slot_tok, valid))
    return in_maps, slot_maps


def kernel(**inputs):
    if "nc" not in _CACHE:
        _CACHE["nc"] = _build_module()
    nc = _CACHE["nc"]
    in_maps, slot_maps = _host_prep(inputs)
    res = run_bass_kernel_spmd(nc, in_maps, core_ids=list(range(8)),
                               **_CACHE.get("run_kwargs", {}))
    _CACHE["last"] = res
    B, K, R = inputs["top_indices"].shape
    out = np.zeros((B, K * R, D), np.float32)
    for q in range(8):
        b = q // 4
        oT = np.asarray(res.results[q]["outT"], np.float32)
        slot_tok, valid = slot_maps[q]
        out[b, slot_tok[valid]] = oT.T[valid]
    return out.reshape(B, K, R, D)


# revision 11
# speedup vs baseline: 1.1312x; 1.1312x over previous
"""Trainium2 Bass kernel for nn_DeformableRead (deformable attention read).

8 NeuronCores SPMD: core q -> batch q//4, anchor-cell rows 8*(q%4)..+8 (256
cells). Tokens routed to the core owning their anchor cell and densely packed
into 8-slot groups (a cell with n tokens occupies ceil(n/8) groups); 16 groups
per 128-slot chunk. Sample points live in fixed windows around each anchor
cell (9x9/5x5/4x4 at L2/L3/L4); bilinear sampling over a window is a dense
122-tap PE contraction (zero-padded to 128 taps) with separable hat weights
relu(1-|xi-i|) -- gather-free. Offsets/logits are produced row-major
(slot-partition) via a transposed wda matmul with LN affine + biases folded
into the weights; softmax and hat/kappa construction split across
Vector/GpSimd/Scalar engines. Host does layout only.
"""

import numpy as np
import ml_dtypes

import concourse.bass as bass
import concourse.bacc as bacc
import concourse.tile as tile
from concourse import mybir
from concourse.bass_utils import run_bass_kernel_spmd

D, H, NL, M = 192, 6, 3, 4
NF = 8
SIGMAS = (4.0, 2.0, 1.0)
WXY = (9, 5, 4)
CLO = (4.0, 2.0, 1.5)
PADL = (2, 1, 1)
SCALE = (4, 2, 1)
G8 = 8                    # slots per group
CPC = 16                  # groups per chunk
NCH = 24                  # chunks per core (asserted in host prep)
S = NCH * 128             # 3072 slots
NCS = S // 512            # phase-A chunks
KW = 128                  # padded tap count
WSQ = (81, 25, 16)
WSQ2 = (82, 26, 16)       # even-padded tap blocks
LOFF2 = (0, 82, 108)      # tap offset per level in padded 128-tap space
TOFF2 = (0, 6 * 4 * 82, 6 * 4 * 82 + 6 * 4 * 26)   # (0, 1968, 2592)
TMPW = TOFF2[2] + 6 * 4 * 16                        # 2976
SOFF2 = (0, 6 * 2 * 82, 6 * 2 * 82 + 6 * 2 * 26)   # (0, 984, 1296)
SUMW = SOFF2[2] + 6 * 2 * 16                        # 1488
HOFF = (0, 36, 56)        # per-head hat sub-block offsets (x or y block)
HATB = 432                # per-coord hat block (6h * 72)
BF16 = mybir.dt.bfloat16
F32 = mybir.dt.float32

_CACHE = {}


def _ap(base, free_off, dims):
    """Custom AP: base tile slice (sets partition range), explicit free dims."""
    return bass.AP(tensor=base.tensor, offset=base.offset + free_off,
                   ap=[base.ap[0]] + [list(d) for d in dims])


def _build_module():
    nc = bacc.Bacc("TRN2", target_bir_lowering=False, debug=False)
    dt = nc.dram_tensor
    uinT = dt("uinT", [2 * D + 32, S], BF16, kind="ExternalInput")
    pblob = dt("pblob", [NCH, KW, CPC * D], BF16, kind="ExternalInput")
    wu = dt("wu", [2 * D + 32, D], BF16, kind="ExternalInput")
    wub = dt("wub", [D, 1], F32, kind="ExternalInput")
    wda = dt("wda", [D + 1, 216], BF16, kind="ExternalInput")
    sigc = dt("sigc", [128, 144], F32, kind="ExternalInput")
    iota2 = dt("iota2", [128, 2 * HATB], F32, kind="ExternalInput")
    onesw = dt("onesw", [96, 96], BF16, kind="ExternalInput")
    identb = dt("identb", [128, 128], BF16, kind="ExternalInput")
    wo1 = dt("wo1", [128, D], BF16, kind="ExternalInput")
    wo2 = dt("wo2", [64, D], BF16, kind="ExternalInput")
    bo = dt("bo", [D, 1], F32, kind="ExternalInput")
    outT = dt("outT", [D, S], F32, kind="ExternalOutput")

    AF = mybir.ActivationFunctionType
    OP = mybir.AluOpType
    AX = mybir.AxisListType
    V = None  # engine markers resolved below

    with tile.TileContext(nc) as tc:
        with (
            tc.tile_pool(name="const", bufs=1) as cpool,
            tc.tile_pool(name="big", bufs=1) as bpool,
        ):
            _sbn = [0]
            def sb(t_ap, shape, dtype):
                _sbn[0] += 1
                nm = f"cst{_sbn[0]}"
                x = cpool.tile(shape, dtype, tag=nm, name=nm)
                nc.sync.dma_start(x[:], t_ap)
                return x

            s_wu = []
            for kc in range(4):
                k0, k1 = kc * 128, min((kc + 1) * 128, 416)
                s_wu.append(sb(wu[k0:k1, :], [k1 - k0, D], BF16))
            s_wub = [sb(wub[0:96, :], [96, 1], F32), sb(wub[96:192, :], [96, 1], F32)]
            s_wda0 = sb(wda[0:96, :], [96, 216], BF16)
            s_wda1 = sb(wda[96:193, :], [97, 216], BF16)
            s_sigc = sb(sigc[:], [128, 144], F32)
            s_iota = sb(iota2[:], [128, 2 * HATB], F32)
            s_ones = sb(onesw[:], [96, 96], BF16)
            s_idb = sb(identb[:], [128, 128], BF16)
            s_wo1 = sb(wo1[:], [128, D], BF16)
            s_wo2 = sb(wo2[:], [64, D], BF16)
            s_bo = [sb(bo[0:96, :], [96, 1], F32), sb(bo[96:192, :], [96, 1], F32)]
            s_eps = cpool.tile([96, 1], F32, name="s_eps")
            nc.vector.memset(s_eps[:], 1e-5)

            u_r = [bpool.tile([96, S], BF16, tag="ur0", name="ur0"),
                   bpool.tile([97, S], BF16, tag="ur1", name="ur1")]
            nc.vector.memset(u_r[1][96:97, :], 1.0)
            ymS = [bpool.tile([96, S], BF16, tag="ym0", name="ym0"),
                   bpool.tile([96, S], BF16, tag="ym1", name="ym1")]
            varS = bpool.tile([96, S], F32, tag="varS", name="varS")
            rrS = bpool.tile([96, S], BF16, tag="rrS", name="rrS")

            # ======== phase A: u = gelu(W_u @ u_in); LN stats ========
            with (
                tc.tile_pool(name="wk", bufs=2) as wpool,
                tc.tile_pool(name="ucp", bufs=2) as ucpool,
                tc.tile_pool(name="psU", bufs=2, space="PSUM") as psU,
                tc.tile_pool(name="psB", bufs=2, space="PSUM") as psB,
            ):
                for ci in range(NCS):
                    n0 = ci * 512
                    uc = ucpool.tile([128, 4, 512], BF16, tag="uc")
                    for kc in range(4):
                        k0, k1 = kc * 128, min((kc + 1) * 128, 416)
                        nc.sync.dma_start(uc[:k1 - k0, kc, :],
                                          uinT[k0:k1, n0:n0 + 512])
                    y = []
                    for mc in range(2):
                        pu = psU.tile([96, 512], F32, tag="pu")
                        for kc in range(4):
                            kk = min(128, 416 - kc * 128)
                            nc.tensor.matmul(
                                pu[:],
                                s_wu[kc][:, mc * 96:(mc + 1) * 96],
                                uc[:kk, kc, :],
                                start=(kc == 0), stop=(kc == 3))
                        yt = wpool.tile([96, 512], BF16, tag=f"y{mc}", name=f"y{mc}")
                        nc.scalar.activation(out=yt[:], in_=pu[:],
                                             func=AF.Gelu, bias=s_wub[mc],
                                             scale=1.0)
                        y.append(yt)
                    y2 = []
                    for mc in range(2):
                        y2t = wpool.tile([96, 512], BF16, tag=f"y2{mc}", name=f"y2{mc}")
                        nc.vector.tensor_mul(y2t[:], y[mc][:], y[mc][:])
                        y2.append(y2t)
                    pst = psB.tile([96, 2, 512], F32, tag="pst")
                    for st, srcs in ((0, y), (1, y2)):
                        for kc in range(2):
                            nc.tensor.matmul(
                                pst[:, st, :], s_ones[:], srcs[kc][:],
                                start=(kc == 0), stop=(kc == 1))
                    mu = wpool.tile([96, 512], F32, tag="mu")
                    nc.scalar.mul(mu[:], pst[:, 0, :], 1.0 / D)
                    mu2 = wpool.tile([96, 512], F32, tag="mu2")
                    nc.scalar.activation(out=mu2[:], in_=pst[:, 0, :],
                                         func=AF.Square, scale=1.0 / D)
                    nc.vector.scalar_tensor_tensor(
                        out=varS[:, n0:n0 + 512], in0=pst[:, 1, :],
                        scalar=1.0 / D, in1=mu2[:],
                        op0=OP.mult, op1=OP.subtract)
                    for mc, eng in ((0, nc.gpsimd), (1, nc.vector)):
                        eng.tensor_sub(ymS[mc][:, n0:n0 + 512],
                                       y[mc][:], mu[:])
                # LN epilogue: rr = 1/sqrt(var+eps); u_r = ym * rr
                nc.scalar.activation(out=rrS[:], in_=varS[:],
                                     func=AF.Abs_reciprocal_sqrt,
                                     bias=s_eps, scale=1.0)
                for mc in range(2):
                    nc.vector.tensor_mul(u_r[mc][0:96, :], ymS[mc][:], rrS[:])

            # ======== phase B: per 128-slot chunk ========
            with (
                tc.tile_pool(name="fp", bufs=3) as fpool,
                tc.tile_pool(name="tp", bufs=3) as tpool,
                tc.tile_pool(name="kp", bufs=3) as kpool,
                tc.tile_pool(name="pp", bufs=2) as ppool,
                tc.tile_pool(name="psA", bufs=1, space="PSUM") as psA,
                tc.tile_pool(name="psK", bufs=2, space="PSUM") as psK,
                tc.tile_pool(name="psX", bufs=1, space="PSUM") as psX,
                tc.tile_pool(name="psD", bufs=2, space="PSUM") as psD,
            ):
                KB = 3  # kpool bufs
                for q in range(NCH):
                    c0 = q * 128
                    # -- offsets/logits row-major --
                    pda = psA.tile([128, 216], F32, tag="pda")
                    nc.tensor.matmul(pda[:], u_r[0][:, c0:c0 + 128],
                                     s_wda0[:], start=True, stop=False)
                    nc.tensor.matmul(pda[:], u_r[1][:, c0:c0 + 128],
                                     s_wda1[:], start=False, stop=True)
                    th = fpool.tile([128, 144], F32, tag="th")
                    nc.scalar.activation(out=th[:], in_=pda[:, 0:144],
                                         func=AF.Tanh)
                    ex = fpool.tile([128, 72], F32, tag="ex")
                    nc.scalar.activation(out=ex[:], in_=pda[:, 144:216],
                                         func=AF.Exp)
                    ssum = fpool.tile([128, 6], F32, tag="ssum")
                    nc.vector.tensor_reduce(
                        out=ssum[:], in_=_ap(ex[:], 0, [[12, 6], [1, 12]]),
                        axis=AX.X, op=OP.add)
                    rz = fpool.tile([128, 6], F32, tag="rz")
                    nc.vector.reciprocal_approx_fast(out=rz[:], in_=ssum[:])
                    wts = fpool.tile([128, 72], F32, tag="wts")
                    nc.vector.tensor_mul(wts[:], ex[:],
                                         _ap(rz[:], 0, [[1, 6], [0, 12]]))
                    # -- hats --
                    rm2 = fpool.tile([128, 144], F32, tag="rm2")
                    nc.vector.tensor_mul(rm2[:], th[:], s_sigc[:])
                    hxy = fpool.tile([128, 2 * HATB], F32, tag="hxy")
                    for coord, l, eng in ((0, 0, nc.gpsimd), (0, 1, nc.gpsimd),
                                          (0, 2, nc.vector), (1, 0, nc.gpsimd),
                                          (1, 1, nc.gpsimd), (1, 2, nc.vector)):
                        w = WXY[l]
                        eng.tensor_sub(
                            _ap(hxy[:], coord * HATB + HOFF[l],
                                [[72, 6], [w, 4], [1, w]]),
                            _ap(rm2[:], 8 * l + coord,
                                [[24, 6], [2, 4], [0, w]]),
                            _ap(s_iota[:], coord * HATB + HOFF[l],
                                [[72, 6], [w, 4], [1, w]]))
                    nc.scalar.activation(out=hxy[:], in_=hxy[:], func=AF.Abs)
                    nc.scalar.activation(out=hxy[:], in_=hxy[:], func=AF.Relu,
                                         bias=1.0, scale=-1.0)
                    # -- weighted y-hats --
                    hyw = fpool.tile([128, HATB], F32, tag="hyw")
                    for l, eng in ((0, nc.gpsimd), (1, nc.vector),
                                   (2, nc.vector)):
                        w = WXY[l]
                        eng.tensor_mul(
                            _ap(hyw[:], HOFF[l], [[72, 6], [w, 4], [1, w]]),
                            _ap(hxy[:], HATB + HOFF[l],
                                [[72, 6], [w, 4], [1, w]]),
                            _ap(wts[:], 4 * l, [[12, 6], [1, 4], [0, w]]))
                    # -- products --
                    tmp = tpool.tile([128, TMPW], BF16, tag="tmp")
                    for l in range(NL):
                        w = WXY[l]
                        for m in range(4):
                            eng = nc.gpsimd if (l == 0 and m % 2 == 1) else nc.vector
                            eng.tensor_mul(
                                _ap(tmp[:], TOFF2[l] + m * WSQ2[l],
                                    [[4 * WSQ2[l], 6], [w, w], [1, w]]),
                                _ap(hyw[:], HOFF[l] + m * w,
                                    [[72, 6], [1, w], [0, w]]),
                                _ap(hxy[:], HOFF[l] + m * w,
                                    [[72, 6], [0, w], [1, w]]))
                    # -- pairwise adds --
                    sums = fpool.tile([128, SUMW], BF16, tag="sums")
                    for l, eng in ((0, nc.gpsimd), (1, nc.vector),
                                   (2, nc.gpsimd)):
                        w2, w2p = WSQ[l], WSQ2[l]
                        eng.tensor_add(
                            _ap(sums[:], SOFF2[l],
                                [[2 * w2p, 6], [w2p, 2], [1, w2]]),
                            _ap(tmp[:], TOFF2[l],
                                [[4 * w2p, 6], [2 * w2p, 2], [1, w2]]),
                            _ap(tmp[:], TOFF2[l] + w2p,
                                [[4 * w2p, 6], [2 * w2p, 2], [1, w2]]))
                    kap = kpool.tile([128, 6 * KW], BF16, tag="kap")
                    if q < KB:
                        nc.vector.memset(kap[:], 0.0)
                    for l, eng in ((0, nc.gpsimd), (1, nc.vector),
                                   (2, nc.vector)):
                        w2, w2p = WSQ[l], WSQ2[l]
                        eng.tensor_add(
                            _ap(kap[:], LOFF2[l], [[KW, 6], [1, w2]]),
                            _ap(sums[:], SOFF2[l], [[2 * w2p, 6], [1, w2]]),
                            _ap(sums[:], SOFF2[l] + w2p,
                                [[2 * w2p, 6], [1, w2]]))
                    # -- transpose kappa per head -> [tap, h, slot] --
                    pK = psK.tile([KW, 6, 128], BF16, tag="pK")
                    for hh in range(H):
                        nc.tensor.transpose(pK[:, hh, :],
                                            kap[:, hh * KW:(hh + 1) * KW],
                                            s_idb[:])
                    kT = kpool.tile([KW, 6, 128], BF16, tag="kT")
                    nc.vector.tensor_copy(kT[:, 0:3, :], pK[:, 0:3, :])
                    nc.vector.tensor_copy(kT[:, 3:6, :], pK[:, 3:6, :])
                    # -- patch sampling matmuls --
                    patch = ppool.tile([KW, CPC * D], BF16, tag="patch")
                    nc.sync.dma_start(patch[:], pblob[q])
                    pX = psX.tile([128, 1024], F32, tag="pX")
                    pXc = psX.tile([64, 256], F32, tag="pXc")
                    for gg in range(CPC):
                        xo = (gg // 8) * 512 + (gg % 8) * 48
                        nc.tensor.matmul(
                            pX[:, xo:xo + 48],
                            patch[:, gg * D:gg * D + 128],
                            _ap(kT[:], gg * G8, [[128, 6], [1, G8]]),
                            start=True, stop=True)
                        nc.tensor.matmul(
                            pXc[:, gg * 16:(gg + 1) * 16],
                            patch[:, gg * D + 128:(gg + 1) * D],
                            _ap(kT[:], 4 * 128 + gg * G8, [[128, 2], [1, G8]]),
                            start=True, stop=True)
                    # -- head-diagonal selection PSUM->SBUF --
                    XU = kpool.tile([128, 128], BF16, tag="XU")
                    for hh in range(4):
                        src = _ap(pX[32 * hh:32 * hh + 32, :], hh * G8,
                                  [[512, 2], [48, 8], [1, G8]])
                        dst = _ap(XU[32 * hh:32 * hh + 32, :], 0,
                                  [[64, 2], [8, 8], [1, G8]])
                        if hh % 2 == 0:
                            nc.vector.tensor_copy(dst, src)
                        else:
                            nc.scalar.copy(out=dst, in_=src)
                    XL = kpool.tile([64, 128], BF16, tag="XL")
                    for h2 in range(2):
                        src = _ap(pXc[32 * h2:32 * h2 + 32, :], h2 * G8,
                                  [[16, 16], [1, G8]])
                        dst = _ap(XL[32 * h2:32 * h2 + 32, :], 0,
                                  [[8, 16], [1, G8]])
                        if h2 == 0:
                            nc.vector.tensor_copy(dst, src)
                        else:
                            nc.scalar.copy(out=dst, in_=src)
                    # -- output projection --
                    for mc in range(2):
                        pD = psD.tile([96, 128], F32, tag="pD")
                        nc.tensor.matmul(pD[:], s_wo1[:, mc * 96:(mc + 1) * 96],
                                         XU[:], start=True, stop=False)
                        nc.tensor.matmul(pD[:], s_wo2[:, mc * 96:(mc + 1) * 96],
                                         XL[:], start=False, stop=True)
                        od = kpool.tile([96, 128], F32, tag=f"od{mc}",
                                        name=f"od{mc}")
                        nc.scalar.activation(out=od[:], in_=pD[:],
                                             func=AF.Identity, bias=s_bo[mc],
                                             scale=1.0)
                        nc.sync.dma_start(
                            outT[mc * 96:(mc + 1) * 96, c0:c0 + 128], od[:])
    nc.compile()
    return nc


def _host_prep(inputs):
    h = inputs["h"].astype(np.float32)
    ti = inputs["top_indices"].astype(np.int64)
    qc = inputs["query_coords"].astype(np.float32)
    g = inputs["g"].astype(np.float32)
    maps = [np.asarray(inputs["L2_proj"], np.float32),
            np.asarray(inputs["L3_proj"], np.float32),
            np.asarray(inputs["L4_proj"], np.float32)]
    B, K, R = ti.shape

    consts = {}
    consts["wu"] = np.ascontiguousarray(inputs["w_u_w"].T).astype(ml_dtypes.bfloat16)
    consts["wub"] = inputs["w_u_b"].reshape(D, 1).astype(np.float32)
    # wda: [193, 216]; rows 0:192 = (concat(delta, logit) * ln_g).T; row 192 =
    # W @ ln_b + layer bias (consumed via the ones-row of u_r)
    wrows = np.concatenate([inputs["w_delta_w"], inputs["w_a_w"]], 0)  # [216,192]
    brow = (wrows @ inputs["ln_u_b"]
            + np.concatenate([inputs["w_delta_b"], inputs["w_a_b"]]))
    wda = np.concatenate([(wrows * inputs["ln_u_g"][None, :]).T,
                          brow[None, :]], 0)
    consts["wda"] = wda.astype(ml_dtypes.bfloat16)
    sig = np.zeros((H, NL, M, 2), np.float32)
    for l in range(NL):
        sig[:, l] = SIGMAS[l]
    consts["sigc"] = np.tile(sig.reshape(1, 144), (128, 1))
    io = np.zeros((128, 2 * HATB), np.float32)
    for coord in range(2):
        for l in range(NL):
            w = WXY[l]
            for hh in range(H):
                for m in range(M):
                    st = coord * HATB + HOFF[l] + 72 * hh + w * m
                    io[:, st:st + w] = np.arange(w, dtype=np.float32) - CLO[l]
    consts["iota2"] = io
    consts["onesw"] = np.ones((96, 96), ml_dtypes.bfloat16)
    consts["identb"] = np.eye(128, dtype=ml_dtypes.bfloat16)
    woT = np.ascontiguousarray(inputs["w_o_w"].T).astype(np.float32)
    consts["wo1"] = woT[0:128].astype(ml_dtypes.bfloat16)
    consts["wo2"] = woT[128:192].astype(ml_dtypes.bfloat16)
    consts["bo"] = (inputs["w_o_b"] + inputs["e_deform"].reshape(-1)).reshape(D, 1).astype(np.float32)

    pmaps = []
    for b in range(B):
        pm = []
        for l in range(NL):
            Wl = maps[l].shape[3]
            mp = np.transpose(maps[l][b], (1, 2, 0))
            Hp = 32 * SCALE[l] + WXY[l]
            out = np.zeros((Hp, Hp, D), np.float32)
            out[PADL[l]:PADL[l] + Wl, PADL[l]:PADL[l] + Wl] = mp
            pm.append(out)
        pmaps.append(pm)

    freqs = 2.0 ** np.arange(NF, dtype=np.float32)
    NG = NCH * CPC

    in_maps, slot_maps = [], []
    for q in range(8):
        b, crow = q // 4, q % 4
        d = dict(consts)
        cell_of = ti[b].reshape(K * R)
        gcell = np.zeros(NG, np.int64)
        slot_tok = -np.ones(S, np.int64)
        gi = 0
        for cid in range(256):
            toks = np.nonzero(cell_of == crow * 256 + cid)[0]
            for j0 in range(0, len(toks), G8):
                sub = toks[j0:j0 + G8]
                assert gi < NG, "group overflow; raise NCH"
                gcell[gi] = cid
                slot_tok[gi * G8:gi * G8 + len(sub)] = sub
                gi += 1
        # patch blob [NCH, 128, CPC*D]
        ay, ax = gcell // 32, gcell % 32
        pats = np.zeros((NG, KW, D), np.float32)
        for l in range(NL):
            w = WXY[l]
            pm = pmaps[b][l]
            ys = (SCALE[l] * 8 * crow + SCALE[l] * ay)[:, None] + np.arange(w)
            xs = (SCALE[l] * ax)[:, None] + np.arange(w)
            pt = pm[ys[:, :, None], xs[:, None, :], :]      # [NG,w,w,D]
            pats[:, LOFF2[l]:LOFF2[l] + w * w] = pt.reshape(NG, w * w, D)
        d["pblob"] = np.ascontiguousarray(
            pats.reshape(NCH, CPC, KW, D).transpose(0, 2, 1, 3)
                .reshape(NCH, KW, CPC * D)).astype(ml_dtypes.bfloat16)
        valid = slot_tok >= 0
        st = np.where(valid, slot_tok, 0)
        k_of = st // R
        cid_of = cell_of[st]
        h_s = h[b][k_of] * valid[:, None]
        g_s = g[b][cid_of] * valid[:, None]
        qc_s = qc[b][k_of]
        axg = (cid_of % 32).astype(np.float32)
        ayg = (cid_of // 32).astype(np.float32)
        anchor = np.stack([axg * 32 + 16, ayg * 32 + 16], -1)
        dp = (anchor - qc_s) / 1024.0
        xf = dp[:, 0:1] * freqs * 2 * np.pi
        yf = dp[:, 1:2] * freqs * 2 * np.pi
        phi = np.concatenate([np.sin(xf), np.cos(xf), np.sin(yf), np.cos(yf)],
                             -1).astype(np.float32) * valid[:, None]
        u_in = np.concatenate([h_s, g_s, phi], -1)
        d["uinT"] = np.ascontiguousarray(u_in.T).astype(ml_dtypes.bfloat16)
        in_maps.append(d)
        slot_maps.append((slot_tok, valid))
    return in_maps, slot_maps


def kernel(**inputs):
    if "nc" not in _CACHE:
        _CACHE["nc"] = _build_module()
    nc = _CACHE["nc"]
    in_maps, slot_maps = _host_prep(inputs)
    res = run_bass_kernel_spmd(nc, in_maps, core_ids=list(range(8)),
                               **_CACHE.get("run_kwargs", {}))
    _CACHE["last"] = res
    B, K, R = inputs["top_indices"].shape
    out = np.zeros((B, K * R, D), np.float32)
    for q in range(8):
        b = q // 4
        oT = np.asarray(res.results[q]["outT"], np.float32)
        slot_tok, valid = slot_maps[q]
        out[b, slot_tok[valid]] = oT.T[valid]
    return out.reshape(B, K, R, D)
